# revision 10
# baseline (speedup 1.0000x reference)
"""Additive-attention (Bahdanau) layer on 8 TRN2 NeuronCores.

Reference computation (per full batch B=32, T=2048, E=D=H=1024):
    hidden_enc = enc @ W_enc + b_enc                  # [B, T, H]
    hidden_dec = dec @ W_dec + b_dec                  # [B, H]
    logits     = tanh(hidden_enc + hidden_dec) @ w_out + b_out
    probs      = softmax(mask(logits))                # [B, T]
    attn       = einsum('bt,bte->be', probs, enc)     # [B, E]
    return (attn, probs)

Strategy: pure data-parallel over batch (4 batches per core, no collectives).
Host precomputes hidden_dec (+ biases folded) and a transposed copy of enc
(encT) so the big matmul can stream enc with E on partitions without any
on-chip transpose.  b_out is dropped (softmax shift-invariant).  The mask is
applied multiplicatively on exp(logits) which matches the -1e9 fill for any
0/1 mask.  Matmul operands are bf16 (full PE rate); accumulation and all
softmax arithmetic stay fp32.
"""

import ml_dtypes
import numpy as np

BFNP = ml_dtypes.bfloat16

try:
    import concourse  # noqa: F401
except ImportError:
    import sys

    sys.path.insert(0, "/opt/trn_rl_repo")

import concourse.bass as bass  # noqa: E402
import concourse.tile as tile  # noqa: E402
from concourse import bacc, mybir  # noqa: E402
from concourse.bass_utils import run_bass_kernel_spmd  # noqa: E402

F32 = mybir.dt.float32
BF16 = mybir.dt.bfloat16
AF = mybir.ActivationFunctionType
AX = mybir.AxisListType

B, T, E, H = 32, 2048, 1024, 1024
N_CORES = 8
BL = B // N_CORES  # batches per core


def build(tc, B_loc=BL, T_=T, E_=E, H_=H):
    """Per-core Tile program. Same SPMD graph on all cores."""
    nc = tc.nc
    NT = T_ // 512  # 512-wide t tiles
    NTC = T_ // 128  # 128-wide t chunks
    NH = H_ // 128
    NE = E_ // 128
    EW = min(512, E_)  # attn output chunk width
    NEH = E_ // EW

    enc = nc.dram_tensor("enc", [B_loc, T_, E_], BF16, kind="ExternalInput").ap()
    encT = nc.dram_tensor("encT", [B_loc, E_, T_], BF16, kind="ExternalInput").ap()
    w_enc = nc.dram_tensor("w_enc", [E_, H_], BF16, kind="ExternalInput").ap()
    hd = nc.dram_tensor("hd", [128, B_loc * NH], F32, kind="ExternalInput").ap()
    w_out_c = nc.dram_tensor("w_out_c", [128, NH], BF16, kind="ExternalInput").ap()
    mask = nc.dram_tensor("mask", [B_loc, T_], F32, kind="ExternalInput").ap()
    ones1 = nc.dram_tensor("ones1", [1, 1], F32, kind="ExternalInput").ap()
    attn_out = nc.dram_tensor("attn", [B_loc, E_], F32, kind="ExternalOutput").ap()
    probs_out = nc.dram_tensor("probs", [B_loc, T_], F32, kind="ExternalOutput").ap()

    with (
        tc.tile_pool(name="const", bufs=1) as constp,
        tc.tile_pool(name="wp", bufs=1) as wp,
        tc.tile_pool(name="xt", bufs=18) as xtp,
        tc.tile_pool(name="nat", bufs=6) as natp,
        tc.tile_pool(name="th", bufs=4) as thp,
        tc.tile_pool(name="rows", bufs=4) as rowp,
        tc.tile_pool(name="ph", bufs=2, space="PSUM") as php,
        tc.tile_pool(name="pl", bufs=2, space="PSUM") as plp,
        tc.tile_pool(name="pet", bufs=2, space="PSUM") as petp,
        tc.tile_pool(name="pa", bufs=2, space="PSUM") as pap,
    ):
        # ---- persistent constants ----
        w_sb = []
        for ec in range(NE):
            wt = wp.tile([128, H_], BF16, name=f"w{ec}", tag=f"w{ec}")
            nc.sync.dma_start(wt[:], w_enc[ec * 128 : (ec + 1) * 128, :])
            w_sb.append(wt)
        hd_sb = constp.tile([128, B_loc * NH], F32, name="hd_sb", tag="hd")
        nc.sync.dma_start(hd_sb[:], hd[:])
        wo_sb = constp.tile([128, NH], BF16, name="wo_sb", tag="wo")
        nc.sync.dma_start(wo_sb[:], w_out_c[:])
        mk_sb = []
        for b in range(B_loc):
            mt = constp.tile([1, T_], F32, name=f"mk{b}", tag=f"mk{b}")
            nc.sync.dma_start(mt[:], mask[b : b + 1, :])
            mk_sb.append(mt)
        on1 = constp.tile([1, 1], F32, name="on1", tag="on1")
        nc.sync.dma_start(on1[:], ones1[:])

        # PE work that must slot between later matmul groups to avoid PE
        # stalls: each entry is a closure issuing one TensorE instruction.
        pe_pending = []

        def drain_pe(n):
            for _ in range(min(n, len(pe_pending))):
                pe_pending.pop(0)()

        for b in range(B_loc):
            pet = petp.tile([128, NTC], F32, name=f"pet{b}", tag="pet")
            zp = rowp.tile([1, NT], F32, name=f"zp{b}", tag="zp")
            erm_tiles = []
            for tt in range(NT):
                t0 = tt * 512
                xts = []
                for ec in range(NE):
                    x = xtp.tile([128, 512], BF16, name=f"x{b}_{tt}_{ec}", tag="xt")
                    nc.sync.dma_start(
                        x[:], encT[b, ec * 128 : (ec + 1) * 128, t0 : t0 + 512]
                    )
                    xts.append(x)
                pl = plp.tile([1, 512], F32, name=f"pl{b}_{tt}", tag="pl")
                for hc in range(NH):
                    ph = php.tile([128, 512], F32, name=f"ph{b}_{tt}_{hc}", tag="ph")
                    for ec in range(NE):
                        nc.tensor.matmul(
                            ph[:],
                            w_sb[ec][:, hc * 128 : (hc + 1) * 128],
                            xts[ec][:],
                            start=(ec == 0),
                            stop=(ec == NE - 1),
                        )
                    th = thp.tile([128, 512], BF16, name=f"th{b}_{tt}_{hc}", tag="th")
                    nc.scalar.activation(
                        th[:],
                        ph[:],
                        AF.Tanh,
                        bias=hd_sb[:, b * NH + hc : b * NH + hc + 1],
                    )
                    # logits contribution of this h-chunk; deferred by one
                    # hidden group so the PE never waits on the tanh.
                    pe_pending.append(
                        (
                            lambda pl=pl, hc=hc, th=th: nc.tensor.matmul(
                                pl[:],
                                wo_sb[:, hc : hc + 1],
                                th[:],
                                start=(hc == 0),
                                stop=(hc == NH - 1),
                            )
                        )
                    )
                    drain_pe(1 if (tt, hc) != (0, 0) else 0)
                er = rowp.tile([1, 512], F32, name=f"er{b}_{tt}", tag="er")
                erm = rowp.tile(
                    [1, 512], F32, name=f"erm{b}_{tt}", tag="erm", bufs=NT + 1
                )

                def epilogue(pl=pl, er=er, erm=erm, b=b, t0=t0, tt=tt, zp=zp):
                    nc.scalar.activation(er[:], pl[:], AF.Exp)
                    nc.vector.tensor_mul(
                        erm[:], er[:], mk_sb[b][:, t0 : t0 + 512]
                    )
                    nc.vector.reduce_sum(zp[:, tt : tt + 1], erm[:], axis=AX.X)

                # transposes of the masked-exp row into pet columns (PE work,
                # deferred behind later hidden groups)
                first = [True]

                def transpose_col(pet=pet, erm=erm, c0=tt * 4, ep=epilogue, first=first):
                    if first[0]:
                        ep()
                        first[0] = False
                    for k in range(4):
                        nc.tensor.transpose(
                            pet[:, c0 + k : c0 + k + 1],
                            erm[:, k * 128 : (k + 1) * 128],
                            on1[:],
                        )

                pe_pending.append(transpose_col)
                erm_tiles.append(erm)
            # flush deferred PE work for this batch
            drain_pe(len(pe_pending))

            # softmax normalizer
            z = rowp.tile([1, 1], F32, name=f"z{b}", tag="z")
            nc.vector.reduce_sum(z[:], zp[:], axis=AX.X)
            rz = rowp.tile([1, 1], F32, name=f"rz{b}", tag="rz")
            nc.vector.reciprocal(rz[:], z[:])

            # probs output
            for tt in range(NT):
                pr = rowp.tile([1, 512], F32, name=f"pr{b}_{tt}", tag="pr")
                nc.vector.tensor_scalar_mul(pr[:], erm_tiles[tt][:], rz[:])
                nc.sync.dma_start(
                    probs_out[b : b + 1, tt * 512 : (tt + 1) * 512], pr[:]
                )

            # exp columns for the attention matmul
            ec_sb = rowp.tile([128, NTC], BF16, name=f"ec{b}", tag="ec")
            nc.scalar.copy(ec_sb[:], pet[:])

            # attn[e] = sum_t expw[t] * enc[t, e], scaled by rz afterwards
            pa_tiles = [
                pap.tile([1, EW], F32, name=f"pa{b}_{eh}", tag="pa")
                for eh in range(NEH)
            ]
            for tcn in range(NTC):
                natt = natp.tile([128, E_], BF16, name=f"nat{b}_{tcn}", tag="nat")
                nc.sync.dma_start(natt[:], enc[b, tcn * 128 : (tcn + 1) * 128, :])
                for eh in range(NEH):
                    nc.tensor.matmul(
                        pa_tiles[eh][:],
                        ec_sb[:, tcn : tcn + 1],
                        natt[:, eh * EW : (eh + 1) * EW],
                        start=(tcn == 0),
                        stop=(tcn == NTC - 1),
                    )
            for eh in range(NEH):
                ar = rowp.tile([1, EW], F32, name=f"ar{b}_{eh}", tag="ar")
                nc.vector.tensor_scalar_mul(ar[:], pa_tiles[eh][:], rz[:])
                nc.sync.dma_start(
                    attn_out[b : b + 1, eh * EW : (eh + 1) * EW], ar[:]
                )


def _prepare_core_inputs(enc, dec, inp_mask, W_enc, b_enc, W_dec, b_dec, w_out):
    """Host-side sharding + layout prep. Returns in_maps for the 8 cores."""
    enc = np.ascontiguousarray(np.asarray(enc, dtype=np.float32))
    dec = np.asarray(dec, dtype=np.float32)
    inp_mask = np.ascontiguousarray(np.asarray(inp_mask, dtype=np.float32))
    W_enc = np.ascontiguousarray(np.asarray(W_enc, dtype=np.float32))
    W_dec = np.asarray(W_dec, dtype=np.float32)
    b_enc = np.asarray(b_enc, dtype=np.float32)
    b_dec = np.asarray(b_dec, dtype=np.float32)
    w_out = np.asarray(w_out, dtype=np.float32)

    NH = H // 128
    hd_full = dec @ W_dec + b_dec + b_enc  # [B, H]
    ones1 = np.ones((1, 1), dtype=np.float32)
    w_out_c = np.ascontiguousarray(w_out.reshape(NH, 128).T.astype(BFNP))
    W_enc_bf = np.ascontiguousarray(W_enc.astype(BFNP))
    enc_bf = enc.astype(BFNP)

    in_maps = []
    for c in range(N_CORES):
        sl = slice(c * BL, (c + 1) * BL)
        enc_s = np.ascontiguousarray(enc_bf[sl])  # [BL, T, E]
        encT_s = np.ascontiguousarray(enc_bf[sl].transpose(0, 2, 1))  # [BL, E, T]
        # hd as per-partition columns: hd_cols[p, b*NH+hc] = hd_full[b, hc*128+p]
        hd_cols = np.ascontiguousarray(
            hd_full[sl].reshape(BL, NH, 128).transpose(2, 0, 1).reshape(128, BL * NH)
        )
        in_maps.append(
            {
                "enc": enc_s,
                "encT": encT_s,
                "w_enc": W_enc_bf,
                "hd": hd_cols,
                "w_out_c": w_out_c,
                "mask": np.ascontiguousarray(inp_mask[sl]),
                "ones1": ones1,
            }
        )
    return in_maps


def _build_compiled():
    nc = bacc.Bacc(
        "TRN2",
        target_bir_lowering=False,
        debug=False,
        enable_asserts=True,
        num_devices=N_CORES,
    )
    with tile.TileContext(nc) as tc:
        build(tc)
    nc.compile()
    return nc


def kernel(enc, dec, inp_mask, W_enc, b_enc, W_dec, b_dec, w_out, b_out):
    in_maps = _prepare_core_inputs(
        enc, dec, inp_mask, W_enc, b_enc, W_dec, b_dec, w_out
    )
    nc = _build_compiled()
    res = run_bass_kernel_spmd(nc, in_maps, core_ids=list(range(N_CORES)))
    attn = np.concatenate([res.results[c]["attn"] for c in range(N_CORES)], axis=0)
    probs = np.concatenate([res.results[c]["probs"] for c in range(N_CORES)], axis=0)
    return attn, probs
